# revision 31
# baseline (speedup 1.0000x reference)
"""Trainium2 Bass kernel for the attention-LSTM decoder (v2).

Computation (all T positions share the same (h0, c0) state):
  h0 = tanh(eh @ bridge_hW.T);  c0 = tanh(ec @ bridge_cW.T)
  energy = tanh(enc @ key_W.T + h0 @ query_W.T);  scores = energy . energy_W
  alphas = softmax(scores + maskadd);  ctx = alphas @ enc
  gates = W_ihE @ emb[tok] + [W_ctx @ ctx + W_hh @ h0 + b]   (= gc, per batch)
  c = sig(f)*c0 + sig(i)*tanh(g);  h = sig(o)*tanh(c)
  out = preWemb @ emb[tok] + preWh @ h + preWctx @ ctx

Sharding: data-parallel over batch B=128 across 8 cores (NB=16 each).
v2 design notes:
  - all DRAM tensors host-prepacked partition-major -> large contiguous DMAs
  - phase A processed per batch-PAIR with n=512 matmuls, 2-deep software skew
  - softmax per pair on [1,512] rows; ctx assembled via SBUF->SBUF DMA + PE
    transposes (no K=1 matmul soup)
  - gc computed once as gcT [NB, 4H] with n=512 matmuls, then replicated into
    32-aligned row strips; folded into the gates PSUM via tile_position-packed
    k=2 matmuls (zero vector/scalar cost for the bias)
  - embeddings gathered pre-transposed via dma_gather(transpose=True)
  - gates matmul in fp8(e4m3, x8 scale) DoubleRow = 2x PE rate; x64 undone by
    the activation `scale`
  - output written fp16, token-major restored on host
"""

import numpy as np
from contextlib import ExitStack

import concourse.bass as bass
import concourse.mybir as mybir
import concourse.tile as tile
from concourse import bacc
from concourse.bass_utils import run_bass_kernel_spmd
from concourse.masks import make_identity

FP32 = mybir.dt.float32
F16 = mybir.dt.float16
FP8 = mybir.dt.float8e4
I16 = mybir.dt.int16
AF = mybir.ActivationFunctionType
OP = mybir.AluOpType
AX = mybir.AxisListType
DR = mybir.MatmulPerfMode.DoubleRow

P = 128
H = 512
E = 256
TWOH = 1024
FOURH = 2048
S = 256
T = 256
V = 10000
N_CORES = 8
USE_FP8 = True


def build_kernel(nc, NB):
    NP = NB // 2          # batch pairs
    ntok = NB * T         # tokens per core
    NT = ntok // 512      # 512-token tiles (= pairs)

    dt = lambda name, shape, dtype=F16: nc.dram_tensor(
        name, shape, dtype, kind="ExternalInput")

    encT_d = dt("encT", [P, NP * 8 * 512])          # [p,(pair,k,u,s)]
    encp_d = dt("encp", [P, NB * 2 * TWOH])         # [p,(b,c,d)]
    embT_d = dt("embT", [P, 2 * ntok])              # pre-gathered emb[tok]^T
    # rowblob [1, *]: maskadd | hb | cb | bias2
    rowblob_d = dt("rowblob", [1, NB * S + H + H + FOURH])
    # pblob [128, *]: energyW | ind2 | ehT | ecT
    pblob_d = dt("pblob", [P, 4 + 512 + 8 * NB + 8 * NB])
    # wblob [128, *]: bhWT | bcWT | qWT | keyWT
    wblob_d = dt("wblob", [P, 8 * H + 8 * H + 4 * H + 8 * H])
    wihTe8_d = dt("wihTe8", [P, 2 * FOURH], FP8 if USE_FP8 else F16)
    whhT_d = dt("whhT", [P, 4 * FOURH])
    wihTctx_d = dt("wihTctx", [P, 8 * FOURH])
    preWTemb_d = dt("preWTemb", [P, 2 * H])
    preWTh_d = dt("preWTh", [P, 4 * H])
    preWTctx_d = dt("preWTctx", [P, 8 * H])
    out_d = nc.dram_tensor("out", [P, 4 * NT * 512], F16, kind="ExternalOutput")

    GSC = 1.0 / 64.0 if USE_FP8 else 1.0   # undo x8 scaling of both operands

    with ExitStack() as ctx:
        tc = ctx.enter_context(tile.TileContext(nc))

        # ---------- small consts ----------
        const = ctx.enter_context(tc.tile_pool(name="const", bufs=1))
        ones1 = const.tile([1, 1], F16)
        nc.vector.memset(ones1[:], 1.0)
        identity_h = const.tile([P, P], F16)
        make_identity(nc, identity_h[:])
        ident16 = identity_h[0:16, 0:16]
        pblob_sb = const.tile([P, 4 + 512 + 16 * NB], F16)
        nc.sync.dma_start(out=pblob_sb[:], in_=pblob_d[:])
        energyW_sb = pblob_sb[:, 0:4]
        ind2_sb = pblob_sb[:, 4:516]
        ehT_sb = pblob_sb[:, 516:516 + 8 * NB]
        ecT_sb = pblob_sb[:, 516 + 8 * NB:516 + 16 * NB]
        rowblob_sb = const.tile([1, NB * S + 2 * H + FOURH], F16)
        nc.sync.dma_start(out=rowblob_sb[:], in_=rowblob_d[:])
        maskadd_sb = rowblob_sb[0:1, 0:NB * S]
        hb_sb = rowblob_sb[0:1, NB * S:NB * S + H]
        cb_sb = rowblob_sb[0:1, NB * S + H:NB * S + 2 * H]
        bias2_sb = rowblob_sb[0:1, NB * S + 2 * H:]

        # ---------- tiles for gather + resident weights (DMAs emitted later,
        # after the setup-critical loads, to keep the sync FIFO prioritized) --
        emb_pool = ctx.enter_context(tc.tile_pool(name="embp", bufs=1))
        embT = emb_pool.tile([P, 2 * ntok], F16)
        embT3 = embT[:].rearrange("p (c n) -> p c n", c=2)
        if USE_FP8:
            embT8 = emb_pool.tile([P, 2 * ntok], FP8)
            embT8_3 = embT8[:].rearrange("p (c n) -> p c n", c=2)

        resident = ctx.enter_context(tc.tile_pool(name="resident", bufs=1))
        keyWT_sb = resident.tile([P, 8 * H], F16)
        kw3 = keyWT_sb[:].rearrange("p (k h) -> p k h", k=8)
        wihTe8_sb = resident.tile([P, 2 * FOURH], FP8 if USE_FP8 else F16)
        we3 = wihTe8_sb[:].rearrange("p (c h) -> p c h", c=2)
        preWTemb_sb = resident.tile([P, 2 * H], F16)
        pwe3 = preWTemb_sb[:].rearrange("p (c h) -> p c h", c=2)
        preWTh_sb = resident.tile([P, 4 * H], F16)
        pwh3 = preWTh_sb[:].rearrange("p (k h) -> p k h", k=4)
        preWTctx_sb = resident.tile([P, 8 * H], F16)
        pwc3 = preWTctx_sb[:].rearrange("p (k h) -> p k h", k=8)

        # ---------- persistent state ----------
        state = ctx.enter_context(tc.tile_pool(name="state", bufs=1))
        h0T = state.tile([P, 4 * NB], F16)      # [p,(k,b)]
        c0T = state.tile([P, 4 * NB], FP32)
        qprojT = state.tile([P, 4 * NB], F16)
        ctxT = state.tile([P, 8 * NB], F16)
        ctx_all = state.tile([16, TWOH], F16)
        ocT = state.tile([P, 4 * NB], FP32)
        gcT_s = state.tile([16, FOURH], F16)

        # ---------- setup: bridge h0/c0, qproj ----------
        with tc.tile_pool(name="su_w", bufs=1) as suw, \
             tc.tile_pool(name="su_s", bufs=2) as sus, \
             tc.tile_pool(name="su_ps", bufs=2, space="PSUM") as sups, \
             tc.tile_pool(name="su_tp", bufs=2, space="PSUM") as sutp:
            wblob_sb = suw.tile([P, 28 * H], F16, tag="wb")
            nc.sync.dma_start(out=wblob_sb[:], in_=wblob_d[:])
            bhWT_sb = wblob_sb[:, 0:8 * H]
            bcWT_sb = wblob_sb[:, 8 * H:16 * H]
            qWT_sb = wblob_sb[:, 16 * H:20 * H]
            nc.vector.tensor_copy(keyWT_sb[:], wblob_sb[:, 20 * H:28 * H])
            ones16 = suw.tile([1, 16], F16, tag="o16")
            nc.vector.memset(ones16[:], 1.0)

            def bstack(src_sb, wT_sb, brow, dstT, dst_b, name):
                ps = sups.tile([16, H], FP32, tag="ps")
                for k in range(8):
                    nc.tensor.matmul(
                        ps[:], src_sb[:, k * NB:k * NB + 16],
                        wT_sb[:, k * H:(k + 1) * H],
                        start=(k == 0), stop=False)
                nc.tensor.matmul(ps[:], ones16[0:1, :], brow,
                                 start=False, stop=True)
                nc.scalar.activation(dst_b[:], ps[:], AF.Tanh)
                for m in range(4):
                    tp = sutp.tile([P, 16], F16, tag="tp")
                    nc.tensor.transpose(
                        tp[:], dst_b[0:16, m * P:(m + 1) * P], ident16)
                    nc.vector.tensor_copy(dstT[:, m * NB:m * NB + 16], tp[:])

            h0_b = sus.tile([16, H], F16, tag="h0b")
            c0_b = sus.tile([16, H], F16, tag="c0b")
            bstack(ehT_sb, bhWT_sb, hb_sb, h0T, h0_b, "h0")
            bstack(ecT_sb, bcWT_sb, cb_sb, c0T, c0_b, "c0")

            qp = sups.tile([16, H], FP32, tag="ps")
            for k in range(4):
                nc.tensor.matmul(qp[:], h0T[:, k * NB:k * NB + 16],
                                 qWT_sb[:, k * H:(k + 1) * H],
                                 start=(k == 0), stop=(k == 3))
            qp_b = sus.tile([16, H], F16, tag="qpb")
            nc.vector.tensor_copy(qp_b[:], qp[:])
            for m in range(4):
                tp = sutp.tile([P, 16], F16, tag="tp")
                nc.tensor.transpose(
                    tp[:], qp_b[0:16, m * P:(m + 1) * P], ident16,)
                nc.vector.tensor_copy(qprojT[:, m * NB:m * NB + 16], tp[:])


        gcw = ctx.enter_context(tc.tile_pool(name="gc_w", bufs=1))
        whhT_sb = gcw.tile([P, 4 * FOURH], F16, tag="whh")
        wcx_sb = gcw.tile([P, 8 * FOURH], F16, tag="wcx")

        # ---------- phase A: attention, per pair, 2-deep skew ----------
        with tc.tile_pool(name="encTp", bufs=3) as encTp, \
             tc.tile_pool(name="encpp", bufs=4) as encpp, \
             tc.tile_pool(name="enerp", bufs=2) as enerp, \
             tc.tile_pool(name="arow", bufs=2) as arow, \
             tc.tile_pool(name="ps_pk", bufs=2, space="PSUM") as ps_pk, \
             tc.tile_pool(name="ps_sc", bufs=2, space="PSUM") as ps_sc, \
             tc.tile_pool(name="ps_cu", bufs=2, space="PSUM") as ps_cu, \
             tc.tile_pool(name="ps_tp", bufs=2, space="PSUM") as ps_tp:
            stage = {}
            for it in range(NP + 2):
                # --- A1(it): load + keyproj + energy ---
                if it < NP:
                    p_ = it
                    encT_t = encTp.tile([P, 8 * 512], F16, tag="encT")
                    nc.sync.dma_start(
                        out=encT_t[:],
                        in_=encT_d[:, p_ * 8 * 512:(p_ + 1) * 8 * 512])
                    eT3 = encT_t[:].rearrange("p (k s) -> p k s", k=8)
                    encp_t = encpp.tile([P, 4 * TWOH], F16, tag="encp")
                    nc.sync.dma_start(
                        out=encp_t[:],
                        in_=encp_d[:, p_ * 4 * TWOH:(p_ + 1) * 4 * TWOH])
                    ener_t = enerp.tile([P, 4 * 512], F16, tag="ener")
                    for m in range(4):
                        pk = ps_pk.tile([P, 512], FP32, tag="pk")
                        for k in range(8):
                            nc.tensor.matmul(
                                pk[:], kw3[:, k, m * P:(m + 1) * P],
                                eT3[:, k, :], start=(k == 0), stop=(k == 7))
                        for u in range(2):
                            b = 2 * p_ + u
                            nc.scalar.activation(
                                ener_t[:, m * 512 + u * S:m * 512 + (u + 1) * S],
                                pk[:, u * S:(u + 1) * S], AF.Tanh,
                                bias=qprojT[:, (m * NB + b):(m * NB + b + 1)])
                    stage[it] = dict(ener=ener_t, encp=encp_t)
                    if it == 0:
                        nc.scalar.dma_start(out=embT[:], in_=embT_d[:])
                    if it == 1:
                        nc.scalar.dma_start(out=preWTctx_sb[:], in_=preWTctx_d[:])
                        nc.scalar.dma_start(out=wihTe8_sb[:], in_=wihTe8_d[:])
                    if it == 2:
                        nc.scalar.dma_start(out=preWTemb_sb[:], in_=preWTemb_d[:])
                        nc.scalar.dma_start(out=preWTh_sb[:], in_=preWTh_d[:])
                    if it == 5:
                        nc.scalar.dma_start(out=whhT_sb[:], in_=whhT_d[:])
                    if it == 6:
                        nc.scalar.dma_start(out=wcx_sb[:], in_=wihTctx_d[:])

                # --- A2(it-1): scores ---
                j = it - 1
                if 0 <= j < NP:
                    st = stage[j]
                    sc = ps_sc.tile([1, 512], FP32, tag="sc")
                    for m in range(4):
                        nc.tensor.matmul(
                            sc[:], energyW_sb[:, m:m + 1],
                            st["ener"][:, m * 512:(m + 1) * 512],
                            start=(m == 0), stop=(m == 3))
                    st["sc"] = sc

                # --- A3(it-1): softmax (scalar+DVE only) ---
                if 0 <= j < NP:
                    st = stage[j]
                    sm = arow.tile([1, 512], FP32, tag="sm")
                    nc.vector.tensor_tensor(
                        out=sm[:], in0=st["sc"][:],
                        in1=maskadd_sb[0:1, j * 512:(j + 1) * 512], op=OP.add)
                    e_t = arow.tile([1, 512], F16, tag="e")
                    z_t = arow.tile([1, 4], FP32, tag="z")
                    for u in range(2):
                        half = sm[0:1, u * S:(u + 1) * S]
                        nmx = arow.tile([1, 1], FP32, tag=f"nmx{u}")
                        nc.vector.tensor_reduce(nmx[:], half, AX.X, OP.max,
                                                negate=True)
                        nc.scalar.activation(e_t[0:1, u * S:(u + 1) * S], half,
                                             AF.Exp, bias=nmx[0:1, 0:1])
                        nc.vector.tensor_reduce(
                            z_t[0:1, u:u + 1], e_t[0:1, u * S:(u + 1) * S],
                            AX.X, OP.add)
                    rz = arow.tile([1, 4], FP32, tag="rz")
                    nc.vector.reciprocal(rz[0:1, 0:2], z_t[0:1, 0:2])
                    ea = arow.tile([1, 512], F16, tag="ea")
                    for u in range(2):
                        nc.vector.tensor_scalar_mul(
                            ea[0:1, u * S:(u + 1) * S],
                            e_t[0:1, u * S:(u + 1) * S], rz[0:1, u:u + 1])
                    st["ea"] = ea

                # --- A4(it-2): eT transposes + ctx matmul + ctx row ---
                j2 = it - 2
                if j2 >= 0:
                    st = stage.pop(j2)
                    eTt = arow.tile([P, 4], F16, tag="eT")
                    for c4 in range(4):
                        tp = ps_tp.tile([P, 1], F16, tag="tp")
                        nc.tensor.transpose(
                            tp[:], st["ea"][0:1, c4 * P:(c4 + 1) * P], ones1[:])
                        nc.vector.tensor_copy(eTt[:, c4:c4 + 1], tp[:])
                    for u in range(2):
                        b = 2 * j2 + u
                        for n in range(2):
                            cu = ps_cu.tile([1, H], FP32, tag="cu")
                            for c in range(2):
                                nc.tensor.matmul(
                                    cu[:],
                                    eTt[:, (u * 2 + c):(u * 2 + c + 1)],
                                    st["encp"][:, (u * 2 + c) * TWOH + n * H:
                                               (u * 2 + c) * TWOH + (n + 1) * H],
                                    start=(c == 0), stop=(c == 1))
                            ctmp = arow.tile([1, H], F16, tag="ctmp")
                            nc.scalar.copy(ctmp[:], cu[:])
                            nc.sync.dma_start(
                                out=ctx_all[b:b + 1, n * H:(n + 1) * H],
                                in_=ctmp[:])

        if USE_FP8:
            nc.vector.tensor_scalar_mul(embT8[:], embT[:], 8.0)

        # ---------- ctx assembly + gc + ocT ----------
        with tc.tile_pool(name="gc_ps", bufs=1, space="PSUM") as gcps, \
             tc.tile_pool(name="gc_ps2", bufs=2, space="PSUM") as gcps2:
            ones16b = gcw.tile([1, 16], F16, tag="o16")
            nc.vector.memset(ones16b[:], 1.0)

            # transpose ctx_all -> ctxT
            for k in range(8):
                tp = gcps2.tile([P, 16], F16, tag="tp")
                nc.tensor.transpose(
                    tp[:], ctx_all[0:16, k * P:(k + 1) * P], ident16)
                nc.vector.tensor_copy(ctxT[:, k * NB:k * NB + 16], tp[:])

            # gcT [NB, 4H] = [h0; ctx] @ [whh; wctx]^T + b
            gct_ps = gcps.tile([16, FOURH], FP32, tag="gct")
            for n in range(4):
                dst = gct_ps[:, n * H:(n + 1) * H]
                for k in range(4):
                    nc.tensor.matmul(
                        dst, h0T[:, k * NB:k * NB + 16],
                        whhT_sb[:, k * FOURH + n * H:k * FOURH + (n + 1) * H],
                        start=(k == 0), stop=False)
                for k in range(8):
                    nc.tensor.matmul(
                        dst, ctxT[:, k * NB:k * NB + 16],
                        wcx_sb[:, k * FOURH + n * H:k * FOURH + (n + 1) * H],
                        start=False, stop=False)
                nc.tensor.matmul(dst, ones16b[0:1, :],
                                 bias2_sb[0:1, n * H:(n + 1) * H],
                                 start=False, stop=True)
            nc.vector.tensor_scalar_mul(gcT_s[:], gct_ps[:], 1.0 / GSC)

            # ocT: preWctx @ ctx  (per-batch output bias)
            ocp = gcps.tile([16, H], FP32, tag="oc")
            for k in range(8):
                nc.tensor.matmul(ocp[:], ctxT[:, k * NB:k * NB + 16],
                                 pwc3[:, k, :], start=(k == 0), stop=(k == 7))
            oc_b = gcw.tile([16, H], F16, tag="ocb")
            nc.vector.tensor_copy(oc_b[:], ocp[:])
            for m in range(4):
                tp = gcps2.tile([P, 16], F16, tag="tp")
                nc.tensor.transpose(
                    tp[:], oc_b[0:16, m * P:(m + 1) * P], ident16)
                nc.vector.tensor_copy(ocT[:, m * NB:m * NB + 16], tp[:])

        # ---------- phase B: gates + LSTM + output, 2-deep skew ----------
        with tc.tile_pool(name="gat", bufs=2) as gat, \
             tc.tile_pool(name="repp", bufs=3) as repp, \
             tc.tile_pool(name="lstm", bufs=2) as lstm, \
             tc.tile_pool(name="hTp", bufs=3) as hTp, \
             tc.tile_pool(name="outp", bufs=3) as outp, \
             tc.tile_pool(name="ps_g", bufs=5, space="PSUM") as ps_g, \
             tc.tile_pool(name="ps_o", bufs=3, space="PSUM") as ps_o:
            FUNCS = (AF.Sigmoid, AF.Sigmoid, AF.Tanh, AF.Sigmoid)
            stageB = {}
            rep_tiles = {}

            def emit_rep(tt):
                rep_t = repp.tile([P, FOURH], F16, tag="rep")
                for g in range(4):
                    nc.sync.dma_start(out=rep_t[32 * g:32 * g + 2, :],
                                      in_=gcT_s[2 * tt:2 * tt + 2, :])
                rep_tiles[tt] = rep_t

            emit_rep(0)
            for it in range(NT + 1):
                if it + 1 < NT:
                    emit_rep(it + 1)
                # --- B1(it): gates matmuls + activations ---
                if it < NT:
                    tt = it
                    sI_t = gat.tile([P, 4 * 512], F16, tag="sI")
                    sF_t = gat.tile([P, 4 * 512], F16, tag="sF")
                    tG_t = gat.tile([P, 4 * 512], F16, tag="tG")
                    sO_t = gat.tile([P, 4 * 512], F16, tag="sO")
                    sg = dict(sI=sI_t, sF=sF_t, tG=tG_t, sO=sO_t)
                    gtiles = (sg["sI"], sg["sF"], sg["tG"], sg["sO"])
                    for hs in range(4):
                        pgs = []
                        for g in range(4):
                            mg = g * 4 + hs
                            pg = ps_g.tile([P, 512], FP32, tag="pg")
                            if USE_FP8:
                                nc.tensor.matmul(
                                    pg[:], we3[:, :, mg * P:(mg + 1) * P],
                                    embT8_3[:, :, tt * 512:(tt + 1) * 512],
                                    start=True, stop=False, perf_mode=DR)
                            else:
                                for c in range(2):
                                    nc.tensor.matmul(
                                        pg[:], we3[:, c, mg * P:(mg + 1) * P],
                                        embT3[:, c, tt * 512:(tt + 1) * 512],
                                        start=(c == 0), stop=False)
                            pgs.append(pg)
                        for g in range(4):
                            mg = g * 4 + hs
                            nc.tensor.matmul(
                                pgs[g][:],
                                rep_tiles[tt][32 * g:32 * g + 2,
                                              mg * P:(mg + 1) * P],
                                ind2_sb[32 * g:32 * g + 2, :],
                                start=False, stop=True,
                                tile_position=(32 * g, 0))
                        for g in range(4):
                            nc.scalar.activation(
                                gtiles[g][:, hs * 512:(hs + 1) * 512],
                                pgs[g][:], FUNCS[g], scale=GSC)
                    stageB[it] = sg

                # --- B2(it-1): LSTM elementwise ---
                j = it - 1
                if 0 <= j < NT:
                    sg = stageB[j]
                    hT_t = hTp.tile([P, 4 * 512], F16, tag="hT")
                    for hs in range(4):
                        cs = slice(hs * 512, (hs + 1) * 512)
                        tmp = lstm.tile([P, 512], F16, tag="tmp")
                        nc.vector.tensor_tensor(
                            out=tmp[:], in0=sg["sI"][:, cs], in1=sg["tG"][:, cs],
                            op=OP.mult)
                        cc = lstm.tile([P, 512], F16, tag="cc")
                        for u in range(2):
                            b = 2 * j + u
                            us = slice(hs * 512 + u * S, hs * 512 + (u + 1) * S)
                            nc.vector.tensor_scalar_mul(
                                cc[:, u * S:(u + 1) * S], sg["sF"][:, us],
                                c0T[:, hs * NB + b:hs * NB + b + 1])
                        nc.vector.tensor_tensor(out=cc[:], in0=cc[:],
                                                in1=tmp[:], op=OP.add)
                        tanC = lstm.tile([P, 512], F16, tag="tanC")
                        nc.scalar.activation(tanC[:], cc[:], AF.Tanh)
                        nc.gpsimd.tensor_tensor(
                            out=hT_t[:, cs], in0=sg["sO"][:, cs], in1=tanC[:],
                            op=OP.mult)
                    stageB[j]["hT"] = hT_t

                # --- B3(it-1): output projection + store ---
                j2 = it - 1
                if 0 <= j2 < NT:
                    sg = stageB.pop(j2)
                    rep_tiles.pop(j2, None)
                    hT3 = sg["hT"][:].rearrange("p (k n) -> p k n", k=4)
                    for m in range(4):
                        po = ps_o.tile([P, 512], FP32, tag="po")
                        for c in range(2):
                            nc.tensor.matmul(
                                po[:], pwe3[:, c, m * P:(m + 1) * P],
                                embT3[:, c, j2 * 512:(j2 + 1) * 512],
                                start=(c == 0), stop=False)
                        for k in range(4):
                            nc.tensor.matmul(
                                po[:], pwh3[:, k, m * P:(m + 1) * P],
                                hT3[:, k, :], start=False, stop=(k == 3))
                        o_t = outp.tile([P, 512], F16, tag="o")
                        for u in range(2):
                            b = 2 * j2 + u
                            nc.vector.tensor_scalar(
                                out=o_t[:, u * S:(u + 1) * S],
                                in0=po[:, u * S:(u + 1) * S],
                                scalar1=ocT[:, m * NB + b:m * NB + b + 1],
                                scalar2=None, op0=OP.add)
                        nc.sync.dma_start(
                            out=out_d[:, (m * NT + j2) * 512:
                                      (m * NT + j2 + 1) * 512],
                            in_=o_t[:])
    return nc


# ---------------------------------------------------------------------------
# host side
# ---------------------------------------------------------------------------

def _chunk_pm(w, chunks, dtype=np.float16):
    """[chunks*128, n] -> [128, chunks*n] partition-major."""
    k, n = w.shape
    assert k == chunks * P
    return np.ascontiguousarray(
        w.reshape(chunks, P, n).transpose(1, 0, 2).reshape(P, chunks * n)
    ).astype(dtype)


def _fp8(w, scale=8.0):
    import ml_dtypes
    return np.clip(w.astype(np.float32) * scale, -240.0, 240.0).astype(
        ml_dtypes.float8_e4m3)


def prep_inputs(inputs, n_cores=N_CORES):
    f32 = lambda x: np.asarray(x, dtype=np.float32)
    f16 = lambda x: np.asarray(x, dtype=np.float32).astype(np.float16)
    tgt_seq = np.asarray(inputs["tgt_seq"]).astype(np.int64)
    enc = f32(inputs["encoder_output"])
    eh = f32(inputs["encoder_hidden"])[0]
    ec = f32(inputs["encoder_cell"])[0]
    src_pos = np.asarray(inputs["src_pos"])
    W_ih = f32(inputs["W_ih"])
    pre_W = f32(inputs["pre_W"])

    B = tgt_seq.shape[0]
    NB = B // n_cores
    NP = NB // 2

    wihTe = W_ih[:, :E].T                      # [E, 4H]
    wihTe8 = (_fp8(_chunk_pm(wihTe, 2, np.float32)) if USE_FP8
              else _chunk_pm(wihTe, 2))
    ind2 = np.zeros((P, 512), np.float16)
    for g in range(4):
        ind2[32 * g, 0:S] = 1.0
        ind2[32 * g + 1, S:512] = 1.0

    emb16 = f16(inputs["emb"])
    GSCH = 64.0 if USE_FP8 else 1.0
    wblob = np.concatenate([
        _chunk_pm(f32(inputs["bridge_hW"]).T, 8),
        _chunk_pm(f32(inputs["bridge_cW"]).T, 8),
        _chunk_pm(f32(inputs["query_W"]).T, 4),
        _chunk_pm(f32(inputs["key_W"]).T, 8)], axis=1)
    energyW = np.ascontiguousarray(f16(inputs["energy_W"]).reshape(4, P).T)
    shared = dict(
        wblob=wblob,
        wihTe8=wihTe8,
        whhT=_chunk_pm(f32(inputs["W_hh"]).T, 4),
        wihTctx=_chunk_pm(W_ih[:, E:].T, 8),
        preWTemb=_chunk_pm(pre_W[:, :E].T, 2),
        preWTh=_chunk_pm(pre_W[:, E:E + H].T, 4),
        preWTctx=_chunk_pm(pre_W[:, E + H:].T, 8),
    )
    hb_row = f16(inputs["bridge_hb"]).reshape(1, H)
    cb_row = f16(inputs["bridge_cb"]).reshape(1, H)
    bias2_row = ((f32(inputs["b_ih"]) + f32(inputs["b_hh"])) * 1.0).astype(
        np.float16).reshape(1, FOURH)

    in_maps = []
    for i in range(n_cores):
        sl = slice(i * NB, (i + 1) * NB)
        enc16 = enc[sl].astype(np.float16)          # [NB, S, 2H]
        # encT [p, pair, k, u, s]
        encT = enc16.transpose(0, 2, 1).reshape(NB, 8, P, S)     # [b,k,p,s]
        encT = encT.reshape(NP, 2, 8, P, S).transpose(3, 0, 2, 1, 4)
        encT = np.ascontiguousarray(encT.reshape(P, NP * 8 * 512))
        # encp [p, b, c, d]
        encp = enc16.reshape(NB, 2, P, TWOH).transpose(2, 0, 1, 3)
        encp = np.ascontiguousarray(encp.reshape(P, NB * 2 * TWOH))
        # pre-gathered transposed embeddings [p, c, tok]
        idx = tgt_seq[sl].reshape(-1)                            # [ntok]
        te = emb16[idx]                                          # [ntok, E]
        ntok_ = te.shape[0]
        embT_pm = np.ascontiguousarray(
            te.reshape(ntok_, 2, P).transpose(2, 1, 0).reshape(P, 2 * ntok_))
        m = src_pos[sl, 0, :].astype(np.float32)                 # [NB, S]
        maskadd = np.where(m != 0, 0.0, -3e4).astype(np.float16)
        ehT = _chunk_pm(eh[sl].T, 8)                             # [p,(k,b)]
        ecT = _chunk_pm(ec[sl].T, 8)
        rowblob = np.concatenate(
            [maskadd.reshape(1, NB * S), hb_row, cb_row, bias2_row],
            axis=1).astype(np.float16)
        pblob = np.concatenate([energyW, ind2, ehT, ecT], axis=1)
        in_maps.append(dict(
            encT=encT, encp=encp, embT=embT_pm,
            rowblob=rowblob, pblob=pblob, **shared,
        ))
    return in_maps, NB


_CACHED = {}


def _get_nc(NB):
    if NB not in _CACHED:
        nc = bacc.Bacc("TRN2", target_bir_lowering=False, debug=False)
        build_kernel(nc, NB)
        nc.compile()
        _CACHED[NB] = nc
    return _CACHED[NB]


def kernel(**inputs):
    in_maps, NB = prep_inputs(inputs, N_CORES)
    nc = _get_nc(NB)
    res = run_bass_kernel_spmd(nc, in_maps, list(range(N_CORES)))
    B = np.asarray(inputs["tgt_seq"]).shape[0]
    NT = NB * T // 512
    out = np.empty((B, T, H), dtype=np.float32)
    for i in range(N_CORES):
        o = res.results[i]["out"].astype(np.float32)             # [128, 4*NT*512]
        o = o.reshape(P, 4, NT, 2, S)                            # [p,m,tt,u,t]
        o = o.transpose(2, 3, 4, 1, 0).reshape(NB, S, H)         # [b,t,(m,p)]
        out[i * NB:(i + 1) * NB] = o
    return out


# revision 32
# speedup vs baseline: 1.0631x; 1.0631x over previous
"""Trainium2 Bass kernel for the attention-LSTM decoder (v2).

Computation (all T positions share the same (h0, c0) state):
  h0 = tanh(eh @ bridge_hW.T);  c0 = tanh(ec @ bridge_cW.T)
  energy = tanh(enc @ key_W.T + h0 @ query_W.T);  scores = energy . energy_W
  alphas = softmax(scores + maskadd);  ctx = alphas @ enc
  gates = W_ihE @ emb[tok] + [W_ctx @ ctx + W_hh @ h0 + b]   (= gc, per batch)
  c = sig(f)*c0 + sig(i)*tanh(g);  h = sig(o)*tanh(c)
  out = preWemb @ emb[tok] + preWh @ h + preWctx @ ctx

Sharding: data-parallel over batch B=128 across 8 cores (NB=16 each).
v2 design notes:
  - all DRAM tensors host-prepacked partition-major -> large contiguous DMAs
  - phase A processed per batch-PAIR with n=512 matmuls, 2-deep software skew
  - softmax per pair on [1,512] rows; ctx assembled via SBUF->SBUF DMA + PE
    transposes (no K=1 matmul soup)
  - gc computed once as gcT [NB, 4H] with n=512 matmuls, then replicated into
    32-aligned row strips; folded into the gates PSUM via tile_position-packed
    k=2 matmuls (zero vector/scalar cost for the bias)
  - embeddings gathered pre-transposed via dma_gather(transpose=True)
  - gates matmul in fp8(e4m3, x8 scale) DoubleRow = 2x PE rate; x64 undone by
    the activation `scale`
  - output written fp16, token-major restored on host
"""

import numpy as np
from contextlib import ExitStack

import concourse.bass as bass
import concourse.mybir as mybir
import concourse.tile as tile
from concourse import bacc
from concourse.bass_utils import run_bass_kernel_spmd
from concourse.masks import make_identity

FP32 = mybir.dt.float32
F16 = mybir.dt.float16
FP8 = mybir.dt.float8e4
I16 = mybir.dt.int16
AF = mybir.ActivationFunctionType
OP = mybir.AluOpType
AX = mybir.AxisListType
DR = mybir.MatmulPerfMode.DoubleRow

P = 128
H = 512
E = 256
TWOH = 1024
FOURH = 2048
S = 256
T = 256
V = 10000
N_CORES = 8
USE_FP8 = True


def build_kernel(nc, NB):
    NP = NB // 2          # batch pairs
    ntok = NB * T         # tokens per core
    NT = ntok // 512      # 512-token tiles (= pairs)

    dt = lambda name, shape, dtype=F16: nc.dram_tensor(
        name, shape, dtype, kind="ExternalInput")

    encT_d = dt("encT", [P, NP * 8 * 512])          # [p,(pair,k,u,s)]
    encp_d = dt("encp", [P, NB * 2 * TWOH])         # [p,(b,c,d)]
    embT_d = dt("embT", [P, 2 * ntok])              # pre-gathered emb[tok]^T
    # rowblob [1, *]: maskadd | hb | cb | bias2
    rowblob_d = dt("rowblob", [1, NB * S + H + H + FOURH])
    # pblob [128, *]: energyW | ind2 | ehT | ecT
    pblob_d = dt("pblob", [P, 4 + 512 + 8 * NB + 8 * NB])
    # wblob [128, *]: bhWT | bcWT | qWT | keyWT
    wblob_d = dt("wblob", [P, 8 * H + 8 * H + 4 * H + 8 * H])
    wihTe8_d = dt("wihTe8", [P, 2 * FOURH], FP8 if USE_FP8 else F16)
    whhT_d = dt("whhT", [P, 4 * FOURH])
    wihTctx_d = dt("wihTctx", [P, 8 * FOURH])
    preWTemb_d = dt("preWTemb", [P, 2 * H])
    preWTh_d = dt("preWTh", [P, 4 * H])
    preWTctx_d = dt("preWTctx", [P, 8 * H])
    out_d = nc.dram_tensor("out", [P, 4 * NT * 512], F16, kind="ExternalOutput")

    GSC = 1.0 / 64.0 if USE_FP8 else 1.0   # undo x8 scaling of both operands

    with ExitStack() as ctx:
        tc = ctx.enter_context(tile.TileContext(nc))

        # ---------- small consts ----------
        const = ctx.enter_context(tc.tile_pool(name="const", bufs=1))
        ones1 = const.tile([1, 1], F16)
        nc.vector.memset(ones1[:], 1.0)
        identity_h = const.tile([P, P], F16)
        make_identity(nc, identity_h[:])
        ident16 = identity_h[0:16, 0:16]
        pblob_sb = const.tile([P, 4 + 512 + 16 * NB], F16)
        nc.sync.dma_start(out=pblob_sb[:], in_=pblob_d[:])
        energyW_sb = pblob_sb[:, 0:4]
        ind2_sb = pblob_sb[:, 4:516]
        ehT_sb = pblob_sb[:, 516:516 + 8 * NB]
        ecT_sb = pblob_sb[:, 516 + 8 * NB:516 + 16 * NB]
        rowblob_sb = const.tile([1, NB * S + 2 * H + FOURH], F16)
        nc.sync.dma_start(out=rowblob_sb[:], in_=rowblob_d[:])
        maskadd_sb = rowblob_sb[0:1, 0:NB * S]
        hb_sb = rowblob_sb[0:1, NB * S:NB * S + H]
        cb_sb = rowblob_sb[0:1, NB * S + H:NB * S + 2 * H]
        bias2_sb = rowblob_sb[0:1, NB * S + 2 * H:]

        # ---------- tiles for gather + resident weights (DMAs emitted later,
        # after the setup-critical loads, to keep the sync FIFO prioritized) --
        emb_pool = ctx.enter_context(tc.tile_pool(name="embp", bufs=1))
        embT = emb_pool.tile([P, 2 * ntok], F16)
        embT3 = embT[:].rearrange("p (c n) -> p c n", c=2)
        if USE_FP8:
            embT8 = emb_pool.tile([P, 2 * ntok], FP8)
            embT8_3 = embT8[:].rearrange("p (c n) -> p c n", c=2)

        resident = ctx.enter_context(tc.tile_pool(name="resident", bufs=1))
        keyWT_sb = resident.tile([P, 8 * H], F16)
        kw3 = keyWT_sb[:].rearrange("p (k h) -> p k h", k=8)
        wihTe8_sb = resident.tile([P, 2 * FOURH], FP8 if USE_FP8 else F16)
        we3 = wihTe8_sb[:].rearrange("p (c h) -> p c h", c=2)
        preWTemb_sb = resident.tile([P, 2 * H], F16)
        pwe3 = preWTemb_sb[:].rearrange("p (c h) -> p c h", c=2)
        preWTh_sb = resident.tile([P, 4 * H], F16)
        pwh3 = preWTh_sb[:].rearrange("p (k h) -> p k h", k=4)
        preWTctx_sb = resident.tile([P, 8 * H], F16)
        pwc3 = preWTctx_sb[:].rearrange("p (k h) -> p k h", k=8)

        # ---------- persistent state ----------
        state = ctx.enter_context(tc.tile_pool(name="state", bufs=1))
        h0T = state.tile([P, 4 * NB], F16)      # [p,(k,b)]
        c0T = state.tile([P, 4 * NB], FP32)
        qprojT = state.tile([P, 4 * NB], F16)
        ctxT = state.tile([P, 8 * NB], F16)
        ctx_all = state.tile([16, TWOH], F16)
        ocT = state.tile([P, 4 * NB], FP32)
        gcT_s = state.tile([16, FOURH], F16)

        # ---------- setup: bridge h0/c0, qproj ----------
        with tc.tile_pool(name="su_w", bufs=1) as suw, \
             tc.tile_pool(name="su_s", bufs=2) as sus, \
             tc.tile_pool(name="su_ps", bufs=2, space="PSUM") as sups, \
             tc.tile_pool(name="su_tp", bufs=2, space="PSUM") as sutp:
            wblob_sb = suw.tile([P, 28 * H], F16, tag="wb")
            nc.sync.dma_start(out=wblob_sb[:], in_=wblob_d[:])
            bhWT_sb = wblob_sb[:, 0:8 * H]
            bcWT_sb = wblob_sb[:, 8 * H:16 * H]
            qWT_sb = wblob_sb[:, 16 * H:20 * H]
            nc.vector.tensor_copy(keyWT_sb[:], wblob_sb[:, 20 * H:28 * H])
            ones16 = suw.tile([1, 16], F16, tag="o16")
            nc.vector.memset(ones16[:], 1.0)

            def bstack(src_sb, wT_sb, brow, dstT, dst_b, name):
                ps = sups.tile([16, H], FP32, tag="ps")
                for k in range(8):
                    nc.tensor.matmul(
                        ps[:], src_sb[:, k * NB:k * NB + 16],
                        wT_sb[:, k * H:(k + 1) * H],
                        start=(k == 0), stop=False)
                nc.tensor.matmul(ps[:], ones16[0:1, :], brow,
                                 start=False, stop=True)
                nc.scalar.activation(dst_b[:], ps[:], AF.Tanh)
                for m in range(4):
                    tp = sutp.tile([P, 16], F16, tag="tp")
                    nc.tensor.transpose(
                        tp[:], dst_b[0:16, m * P:(m + 1) * P], ident16)
                    nc.vector.tensor_copy(dstT[:, m * NB:m * NB + 16], tp[:])

            h0_b = sus.tile([16, H], F16, tag="h0b")
            c0_b = sus.tile([16, H], F16, tag="c0b")
            bstack(ehT_sb, bhWT_sb, hb_sb, h0T, h0_b, "h0")
            bstack(ecT_sb, bcWT_sb, cb_sb, c0T, c0_b, "c0")

            qp = sups.tile([16, H], FP32, tag="ps")
            for k in range(4):
                nc.tensor.matmul(qp[:], h0T[:, k * NB:k * NB + 16],
                                 qWT_sb[:, k * H:(k + 1) * H],
                                 start=(k == 0), stop=(k == 3))
            qp_b = sus.tile([16, H], F16, tag="qpb")
            nc.vector.tensor_copy(qp_b[:], qp[:])
            for m in range(4):
                tp = sutp.tile([P, 16], F16, tag="tp")
                nc.tensor.transpose(
                    tp[:], qp_b[0:16, m * P:(m + 1) * P], ident16,)
                nc.vector.tensor_copy(qprojT[:, m * NB:m * NB + 16], tp[:])


        gcw = ctx.enter_context(tc.tile_pool(name="gc_w", bufs=1))
        whhT_sb = gcw.tile([P, 4 * FOURH], F16, tag="whh")
        wcx_sb = gcw.tile([P, 8 * FOURH], F16, tag="wcx")

        # ---------- phase A: attention, per pair, 2-deep skew ----------
        with tc.tile_pool(name="encTp", bufs=3) as encTp, \
             tc.tile_pool(name="encpp", bufs=4) as encpp, \
             tc.tile_pool(name="enerp", bufs=2) as enerp, \
             tc.tile_pool(name="arow", bufs=2) as arow, \
             tc.tile_pool(name="ps_pk", bufs=2, space="PSUM") as ps_pk, \
             tc.tile_pool(name="ps_sc", bufs=2, space="PSUM") as ps_sc, \
             tc.tile_pool(name="ps_cu", bufs=2, space="PSUM") as ps_cu, \
             tc.tile_pool(name="ps_tp", bufs=2, space="PSUM") as ps_tp:
            stage = {}
            for it in range(NP + 2):
                # --- A1(it): load + keyproj + energy ---
                if it < NP:
                    p_ = it
                    encT_t = encTp.tile([P, 8 * 512], F16, tag="encT")
                    nc.sync.dma_start(
                        out=encT_t[:],
                        in_=encT_d[:, p_ * 8 * 512:(p_ + 1) * 8 * 512])
                    eT3 = encT_t[:].rearrange("p (k s) -> p k s", k=8)
                    encp_t = encpp.tile([P, 4 * TWOH], F16, tag="encp")
                    nc.sync.dma_start(
                        out=encp_t[:],
                        in_=encp_d[:, p_ * 4 * TWOH:(p_ + 1) * 4 * TWOH])
                    ener_t = enerp.tile([P, 4 * 512], F16, tag="ener")
                    for m in range(4):
                        pk = ps_pk.tile([P, 512], FP32, tag="pk")
                        for k in range(8):
                            nc.tensor.matmul(
                                pk[:], kw3[:, k, m * P:(m + 1) * P],
                                eT3[:, k, :], start=(k == 0), stop=(k == 7))
                        for u in range(2):
                            b = 2 * p_ + u
                            nc.scalar.activation(
                                ener_t[:, m * 512 + u * S:m * 512 + (u + 1) * S],
                                pk[:, u * S:(u + 1) * S], AF.Tanh,
                                bias=qprojT[:, (m * NB + b):(m * NB + b + 1)])
                    stage[it] = dict(ener=ener_t, encp=encp_t)
                    if it == 0:
                        nc.scalar.dma_start(out=embT[:], in_=embT_d[:])
                    if it == 1:
                        nc.scalar.dma_start(out=preWTctx_sb[:], in_=preWTctx_d[:])
                        nc.scalar.dma_start(out=wihTe8_sb[:], in_=wihTe8_d[:])
                    if it == 2:
                        nc.scalar.dma_start(out=preWTemb_sb[:], in_=preWTemb_d[:])
                        nc.scalar.dma_start(out=preWTh_sb[:], in_=preWTh_d[:])
                    if it == 5:
                        nc.scalar.dma_start(out=whhT_sb[:], in_=whhT_d[:])
                    if it == 6:
                        nc.scalar.dma_start(out=wcx_sb[:], in_=wihTctx_d[:])

                # --- A2(it-1): scores ---
                j = it - 1
                if 0 <= j < NP:
                    st = stage[j]
                    sc = ps_sc.tile([1, 512], FP32, tag="sc")
                    for m in range(4):
                        nc.tensor.matmul(
                            sc[:], energyW_sb[:, m:m + 1],
                            st["ener"][:, m * 512:(m + 1) * 512],
                            start=(m == 0), stop=(m == 3))
                    st["sc"] = sc

                # --- A3(it-1): softmax (scalar+DVE only) ---
                if 0 <= j < NP:
                    st = stage[j]
                    sm = arow.tile([1, 512], FP32, tag="sm")
                    nc.vector.tensor_tensor(
                        out=sm[:], in0=st["sc"][:],
                        in1=maskadd_sb[0:1, j * 512:(j + 1) * 512], op=OP.add)
                    e_t = arow.tile([1, 512], F16, tag="e")
                    z_t = arow.tile([1, 4], FP32, tag="z")
                    for u in range(2):
                        half = sm[0:1, u * S:(u + 1) * S]
                        nmx = arow.tile([1, 1], FP32, tag=f"nmx{u}")
                        nc.vector.tensor_reduce(nmx[:], half, AX.X, OP.max,
                                                negate=True)
                        nc.scalar.activation(e_t[0:1, u * S:(u + 1) * S], half,
                                             AF.Exp, bias=nmx[0:1, 0:1])
                        nc.vector.tensor_reduce(
                            z_t[0:1, u:u + 1], e_t[0:1, u * S:(u + 1) * S],
                            AX.X, OP.add)
                    rz = arow.tile([1, 4], FP32, tag="rz")
                    nc.vector.reciprocal(rz[0:1, 0:2], z_t[0:1, 0:2])
                    ea = arow.tile([1, 512], F16, tag="ea")
                    for u in range(2):
                        nc.vector.tensor_scalar_mul(
                            ea[0:1, u * S:(u + 1) * S],
                            e_t[0:1, u * S:(u + 1) * S], rz[0:1, u:u + 1])
                    st["ea"] = ea

                # --- A4(it-2): eT transposes + ctx matmul + ctx row ---
                j2 = it - 2
                if j2 >= 0:
                    st = stage.pop(j2)
                    eTt = arow.tile([P, 4], F16, tag="eT")
                    for c4 in range(4):
                        tp = ps_tp.tile([P, 1], F16, tag="tp")
                        nc.tensor.transpose(
                            tp[:], st["ea"][0:1, c4 * P:(c4 + 1) * P], ones1[:])
                        nc.vector.tensor_copy(eTt[:, c4:c4 + 1], tp[:])
                    for u in range(2):
                        b = 2 * j2 + u
                        for n in range(2):
                            cu = ps_cu.tile([1, H], FP32, tag="cu")
                            for c in range(2):
                                nc.tensor.matmul(
                                    cu[:],
                                    eTt[:, (u * 2 + c):(u * 2 + c + 1)],
                                    st["encp"][:, (u * 2 + c) * TWOH + n * H:
                                               (u * 2 + c) * TWOH + (n + 1) * H],
                                    start=(c == 0), stop=(c == 1))
                            ctmp = arow.tile([1, H], F16, tag="ctmp")
                            nc.scalar.copy(ctmp[:], cu[:])
                            nc.sync.dma_start(
                                out=ctx_all[b:b + 1, n * H:(n + 1) * H],
                                in_=ctmp[:])

        if USE_FP8:
            nc.vector.tensor_scalar_mul(embT8[:], embT[:], 8.0)

        # ---------- ctx assembly + gc + ocT ----------
        with tc.tile_pool(name="gc_ps", bufs=1, space="PSUM") as gcps, \
             tc.tile_pool(name="gc_ps2", bufs=2, space="PSUM") as gcps2:
            ones16b = gcw.tile([1, 16], F16, tag="o16")
            nc.vector.memset(ones16b[:], 1.0)

            # transpose ctx_all -> ctxT
            for k in range(8):
                tp = gcps2.tile([P, 16], F16, tag="tp")
                nc.tensor.transpose(
                    tp[:], ctx_all[0:16, k * P:(k + 1) * P], ident16)
                nc.vector.tensor_copy(ctxT[:, k * NB:k * NB + 16], tp[:])

            # gcT [NB, 4H] = [h0; ctx] @ [whh; wctx]^T + b
            gct_ps = gcps.tile([16, FOURH], FP32, tag="gct")
            for n in range(4):
                dst = gct_ps[:, n * H:(n + 1) * H]
                for k in range(4):
                    nc.tensor.matmul(
                        dst, h0T[:, k * NB:k * NB + 16],
                        whhT_sb[:, k * FOURH + n * H:k * FOURH + (n + 1) * H],
                        start=(k == 0), stop=False)
                for k in range(8):
                    nc.tensor.matmul(
                        dst, ctxT[:, k * NB:k * NB + 16],
                        wcx_sb[:, k * FOURH + n * H:k * FOURH + (n + 1) * H],
                        start=False, stop=False)
                nc.tensor.matmul(dst, ones16b[0:1, :],
                                 bias2_sb[0:1, n * H:(n + 1) * H],
                                 start=False, stop=True)
            nc.vector.tensor_scalar_mul(gcT_s[:], gct_ps[:], 1.0 / GSC)

            # ocT: preWctx @ ctx  (per-batch output bias)
            ocp = gcps.tile([16, H], FP32, tag="oc")
            for k in range(8):
                nc.tensor.matmul(ocp[:], ctxT[:, k * NB:k * NB + 16],
                                 pwc3[:, k, :], start=(k == 0), stop=(k == 7))
            oc_b = gcw.tile([16, H], F16, tag="ocb")
            nc.vector.tensor_copy(oc_b[:], ocp[:])
            for m in range(4):
                tp = gcps2.tile([P, 16], F16, tag="tp")
                nc.tensor.transpose(
                    tp[:], oc_b[0:16, m * P:(m + 1) * P], ident16)
                nc.vector.tensor_copy(ocT[:, m * NB:m * NB + 16], tp[:])

        # ---------- phase B: gates + LSTM + output, 2-deep skew ----------
        with tc.tile_pool(name="gat", bufs=2) as gat, \
             tc.tile_pool(name="repp", bufs=3) as repp, \
             tc.tile_pool(name="lstm", bufs=2) as lstm, \
             tc.tile_pool(name="hTp", bufs=3) as hTp, \
             tc.tile_pool(name="outp", bufs=3) as outp, \
             tc.tile_pool(name="ps_g", bufs=5, space="PSUM") as ps_g, \
             tc.tile_pool(name="ps_o", bufs=3, space="PSUM") as ps_o:
            FUNCS = (AF.Sigmoid, AF.Sigmoid, AF.Tanh, AF.Sigmoid)
            stageB = {}
            rep_tiles = {}

            def emit_rep(tt):
                rep_t = repp.tile([P, FOURH], F16, tag="rep")
                for g in range(4):
                    nc.sync.dma_start(out=rep_t[32 * g:32 * g + 2, :],
                                      in_=gcT_s[2 * tt:2 * tt + 2, :])
                rep_tiles[tt] = rep_t

            emit_rep(0)
            for it in range(NT + 1):
                if it + 1 < NT:
                    emit_rep(it + 1)
                # --- B1(it): gates matmuls + activations ---
                if it < NT:
                    tt = it
                    sI_t = gat.tile([P, 4 * 512], F16, tag="sI")
                    sF_t = gat.tile([P, 4 * 512], F16, tag="sF")
                    tG_t = gat.tile([P, 4 * 512], F16, tag="tG")
                    sO_t = gat.tile([P, 4 * 512], F16, tag="sO")
                    sg = dict(sI=sI_t, sF=sF_t, tG=tG_t, sO=sO_t)
                    gtiles = (sg["sI"], sg["sF"], sg["tG"], sg["sO"])
                    for hs in range(4):
                        pgs = []
                        for g in range(4):
                            mg = g * 4 + hs
                            pg = ps_g.tile([P, 512], FP32, tag="pg")
                            if USE_FP8:
                                nc.tensor.matmul(
                                    pg[:], we3[:, :, mg * P:(mg + 1) * P],
                                    embT8_3[:, :, tt * 512:(tt + 1) * 512],
                                    start=True, stop=False, perf_mode=DR)
                            else:
                                for c in range(2):
                                    nc.tensor.matmul(
                                        pg[:], we3[:, c, mg * P:(mg + 1) * P],
                                        embT3[:, c, tt * 512:(tt + 1) * 512],
                                        start=(c == 0), stop=False)
                            pgs.append(pg)
                        for g in range(4):
                            mg = g * 4 + hs
                            nc.tensor.matmul(
                                pgs[g][:],
                                rep_tiles[tt][32 * g:32 * g + 2,
                                              mg * P:(mg + 1) * P],
                                ind2_sb[32 * g:32 * g + 2, :],
                                start=False, stop=True,
                                tile_position=(32 * g, 0))
                        for g in range(4):
                            nc.scalar.activation(
                                gtiles[g][:, hs * 512:(hs + 1) * 512],
                                pgs[g][:], FUNCS[g], scale=GSC)
                    stageB[it] = sg

                # --- B2(it-1): LSTM elementwise ---
                j = it - 1
                if 0 <= j < NT:
                    sg = stageB[j]
                    hT_t = hTp.tile([P, 4 * 512], F16, tag="hT")
                    for hs in range(4):
                        cs = slice(hs * 512, (hs + 1) * 512)
                        tmp = lstm.tile([P, 512], F16, tag="tmp")
                        nc.vector.tensor_tensor(
                            out=tmp[:], in0=sg["sI"][:, cs], in1=sg["tG"][:, cs],
                            op=OP.mult)
                        cc = lstm.tile([P, 512], F16, tag="cc")
                        for u in range(2):
                            b = 2 * j + u
                            us = slice(hs * 512 + u * S, hs * 512 + (u + 1) * S)
                            nc.vector.tensor_scalar_mul(
                                cc[:, u * S:(u + 1) * S], sg["sF"][:, us],
                                c0T[:, hs * NB + b:hs * NB + b + 1])
                        nc.vector.tensor_tensor(out=cc[:], in0=cc[:],
                                                in1=tmp[:], op=OP.add)
                        tanC = lstm.tile([P, 512], F16, tag="tanC")
                        nc.scalar.activation(tanC[:], cc[:], AF.Tanh)
                        nc.vector.tensor_tensor(
                            out=hT_t[:, cs], in0=sg["sO"][:, cs], in1=tanC[:],
                            op=OP.mult)
                    stageB[j]["hT"] = hT_t

                # --- B3(it-1): output projection + store ---
                j2 = it - 1
                if 0 <= j2 < NT:
                    sg = stageB.pop(j2)
                    rep_tiles.pop(j2, None)
                    hT3 = sg["hT"][:].rearrange("p (k n) -> p k n", k=4)
                    for m in range(4):
                        po = ps_o.tile([P, 512], FP32, tag="po")
                        for c in range(2):
                            nc.tensor.matmul(
                                po[:], pwe3[:, c, m * P:(m + 1) * P],
                                embT3[:, c, j2 * 512:(j2 + 1) * 512],
                                start=(c == 0), stop=False)
                        for k in range(4):
                            nc.tensor.matmul(
                                po[:], pwh3[:, k, m * P:(m + 1) * P],
                                hT3[:, k, :], start=False, stop=(k == 3))
                        o_t = outp.tile([P, 512], F16, tag="o")
                        for u in range(2):
                            b = 2 * j2 + u
                            nc.vector.tensor_scalar(
                                out=o_t[:, u * S:(u + 1) * S],
                                in0=po[:, u * S:(u + 1) * S],
                                scalar1=ocT[:, m * NB + b:m * NB + b + 1],
                                scalar2=None, op0=OP.add)
                        nc.sync.dma_start(
                            out=out_d[:, (m * NT + j2) * 512:
                                      (m * NT + j2 + 1) * 512],
                            in_=o_t[:])
    return nc


# ---------------------------------------------------------------------------
# host side
# ---------------------------------------------------------------------------

def _chunk_pm(w, chunks, dtype=np.float16):
    """[chunks*128, n] -> [128, chunks*n] partition-major."""
    k, n = w.shape
    assert k == chunks * P
    return np.ascontiguousarray(
        w.reshape(chunks, P, n).transpose(1, 0, 2).reshape(P, chunks * n)
    ).astype(dtype)


def _fp8(w, scale=8.0):
    import ml_dtypes
    return np.clip(w.astype(np.float32) * scale, -240.0, 240.0).astype(
        ml_dtypes.float8_e4m3)


def prep_inputs(inputs, n_cores=N_CORES):
    f32 = lambda x: np.asarray(x, dtype=np.float32)
    f16 = lambda x: np.asarray(x, dtype=np.float32).astype(np.float16)
    tgt_seq = np.asarray(inputs["tgt_seq"]).astype(np.int64)
    enc = f32(inputs["encoder_output"])
    eh = f32(inputs["encoder_hidden"])[0]
    ec = f32(inputs["encoder_cell"])[0]
    src_pos = np.asarray(inputs["src_pos"])
    W_ih = f32(inputs["W_ih"])
    pre_W = f32(inputs["pre_W"])

    B = tgt_seq.shape[0]
    NB = B // n_cores
    NP = NB // 2

    wihTe = W_ih[:, :E].T                      # [E, 4H]
    wihTe8 = (_fp8(_chunk_pm(wihTe, 2, np.float32)) if USE_FP8
              else _chunk_pm(wihTe, 2))
    ind2 = np.zeros((P, 512), np.float16)
    for g in range(4):
        ind2[32 * g, 0:S] = 1.0
        ind2[32 * g + 1, S:512] = 1.0

    emb16 = f16(inputs["emb"])
    GSCH = 64.0 if USE_FP8 else 1.0
    wblob = np.concatenate([
        _chunk_pm(f32(inputs["bridge_hW"]).T, 8),
        _chunk_pm(f32(inputs["bridge_cW"]).T, 8),
        _chunk_pm(f32(inputs["query_W"]).T, 4),
        _chunk_pm(f32(inputs["key_W"]).T, 8)], axis=1)
    energyW = np.ascontiguousarray(f16(inputs["energy_W"]).reshape(4, P).T)
    shared = dict(
        wblob=wblob,
        wihTe8=wihTe8,
        whhT=_chunk_pm(f32(inputs["W_hh"]).T, 4),
        wihTctx=_chunk_pm(W_ih[:, E:].T, 8),
        preWTemb=_chunk_pm(pre_W[:, :E].T, 2),
        preWTh=_chunk_pm(pre_W[:, E:E + H].T, 4),
        preWTctx=_chunk_pm(pre_W[:, E + H:].T, 8),
    )
    hb_row = f16(inputs["bridge_hb"]).reshape(1, H)
    cb_row = f16(inputs["bridge_cb"]).reshape(1, H)
    bias2_row = ((f32(inputs["b_ih"]) + f32(inputs["b_hh"])) * 1.0).astype(
        np.float16).reshape(1, FOURH)

    in_maps = []
    for i in range(n_cores):
        sl = slice(i * NB, (i + 1) * NB)
        enc16 = enc[sl].astype(np.float16)          # [NB, S, 2H]
        # encT [p, pair, k, u, s]
        encT = enc16.transpose(0, 2, 1).reshape(NB, 8, P, S)     # [b,k,p,s]
        encT = encT.reshape(NP, 2, 8, P, S).transpose(3, 0, 2, 1, 4)
        encT = np.ascontiguousarray(encT.reshape(P, NP * 8 * 512))
        # encp [p, b, c, d]
        encp = enc16.reshape(NB, 2, P, TWOH).transpose(2, 0, 1, 3)
        encp = np.ascontiguousarray(encp.reshape(P, NB * 2 * TWOH))
        # pre-gathered transposed embeddings [p, c, tok]
        idx = tgt_seq[sl].reshape(-1)                            # [ntok]
        te = emb16[idx]                                          # [ntok, E]
        ntok_ = te.shape[0]
        embT_pm = np.ascontiguousarray(
            te.reshape(ntok_, 2, P).transpose(2, 1, 0).reshape(P, 2 * ntok_))
        m = src_pos[sl, 0, :].astype(np.float32)                 # [NB, S]
        maskadd = np.where(m != 0, 0.0, -3e4).astype(np.float16)
        ehT = _chunk_pm(eh[sl].T, 8)                             # [p,(k,b)]
        ecT = _chunk_pm(ec[sl].T, 8)
        rowblob = np.concatenate(
            [maskadd.reshape(1, NB * S), hb_row, cb_row, bias2_row],
            axis=1).astype(np.float16)
        pblob = np.concatenate([energyW, ind2, ehT, ecT], axis=1)
        in_maps.append(dict(
            encT=encT, encp=encp, embT=embT_pm,
            rowblob=rowblob, pblob=pblob, **shared,
        ))
    return in_maps, NB


_CACHED = {}


def _get_nc(NB):
    if NB not in _CACHED:
        nc = bacc.Bacc("TRN2", target_bir_lowering=False, debug=False)
        build_kernel(nc, NB)
        nc.compile()
        _CACHED[NB] = nc
    return _CACHED[NB]


def kernel(**inputs):
    in_maps, NB = prep_inputs(inputs, N_CORES)
    nc = _get_nc(NB)
    res = run_bass_kernel_spmd(nc, in_maps, list(range(N_CORES)))
    B = np.asarray(inputs["tgt_seq"]).shape[0]
    NT = NB * T // 512
    out = np.empty((B, T, H), dtype=np.float32)
    for i in range(N_CORES):
        o = res.results[i]["out"].astype(np.float32)             # [128, 4*NT*512]
        o = o.reshape(P, 4, NT, 2, S)                            # [p,m,tt,u,t]
        o = o.transpose(2, 3, 4, 1, 0).reshape(NB, S, H)         # [b,t,(m,p)]
        out[i * NB:(i + 1) * NB] = o
    return out
